# revision 30
# baseline (speedup 1.0000x reference)
"""Fused masked-softmax attention (DotProductAttention) for 8 TRN2 NeuronCores.

Problem: B=16 batches of Q[2048,64] @ K[2048,64]^T -> mask cols >= valid_len
to -1e6 -> softmax -> @ V[2048,64].

Work decomposition: each batch splits into 4 q-quarters of 512 rows (one
PSUM-bank-wide q-tile each) -> 64 independent units.  Units are sorted by
valid k-tile count nv = ceil(valid_len/128) and dealt into 8 SPMD slots of
8 units (one per core); the compiled program runs slot s with a static
nv_s = max over that slot's units.  K-tiles wholly past a unit's valid_len
contribute exactly 0 (the mask row drives exp to underflow), so the extra
tiles cores run inside a slot are harmless and skipped tiles are exact.
For uniform-random valid_lens this cuts total work to ~50-60% of dense;
worst case (all full) equals the dense kernel.

Per-unit kernel (all on-chip, scores never touch HBM):
  * Layout: S^T[k, q] so softmax's k-reduction becomes a matmul and the
    attn @ V contraction needs no transpose of the big matrix.
  * mm1:  S^T chunk [128k, 512q] = kTa[:, ktile].T @ qTa with AUGMENTED
    bf16 operands: kTa = [K^T; mask_row] (65 rows), qTa = [Q^T; ones].
    The 65th contraction row adds -8e6 to every masked column, so masking
    costs zero instructions.  bf16 streams 1 row/cycle on the PE (fp32 is
    4x, fp32r 2x).
  * exp:  ACT engine (the bottleneck, 1 elem/lane/cycle @1.2GHz),
    exp(0.125 * x) straight out of PSUM in merged N<=1536 activations
    (per-instruction overhead ~350 cycles), bf16 out.
  * mm2:  O^T_aug [65, 512q] = sum_k Vaug[ktile].T @ expS^T[ktile] with
    Vaug = [V | ones] (bf16) -> row 64 accumulates the softmax denominator
    in fp32 PSUM.  Interleaved group-by-group with mm1/exp.
  * finish: copy PSUM->SBUF (f32), PE-transpose 128-col chunks back to
    [q, d] layout, reciprocal of denominator column, per-partition scale,
    one merged DMA per unit.
"""

import functools

import numpy as np
import ml_dtypes

import concourse.bacc as bacc
import concourse.tile as tile
from concourse import mybir
from concourse import bass_utils
from concourse.masks import make_identity

B, LQ, LKV, D = 16, 2048, 2048, 64
N_CORES = 8
KT = 128            # k-tile (partition dim of S^T)
QT = 512            # q-rows per unit (= PSUM bank free dim)
NKT = LKV // KT     # 16
NSLOT = (B * LQ) // (N_CORES * QT)  # 8 units per core
GROUP = 3           # max k-tiles per PSUM tile / merged activation
MASK_RAW = -8.0e6   # * 0.125 scale == -1e6 (reference MASK_VALUE)
F32 = mybir.dt.float32
BF16 = mybir.dt.bfloat16


def _widths(nv):
    """Split nv k-tiles into activation groups of width <=3, avoiding 1-wide
    groups (measured regression) where possible.  2-wide groups go FIRST so
    each slot's first activation has the shortest possible mm1 prefix
    (shrinks the ACT stall at slot boundaries)."""
    threes, rem = divmod(nv, 3)
    if rem == 0:
        return [3] * threes
    if rem == 2:
        return [2] + [3] * threes
    if threes >= 1:
        return [2, 2] + [3] * (threes - 1)
    return [1]


@functools.lru_cache(maxsize=4)
def _build_module(nv_slots):
    nc = bacc.Bacc(None)
    qta_d = nc.dram_tensor("qta", [NSLOT, D + 1, QT], BF16, kind="ExternalInput")
    kta_d = nc.dram_tensor("kta", [NSLOT, D + 1, LKV], BF16, kind="ExternalInput")
    vau_d = nc.dram_tensor("vaug", [128, NSLOT * NKT * (D + 1)], BF16, kind="ExternalInput")
    out_d = nc.dram_tensor("o", [NSLOT, QT, D], F32, kind="ExternalOutput")

    slot_groups = []
    for nv in nv_slots:
        groups, g = [], 0
        for w in _widths(nv):
            groups.append((g, w))
            g += w
        assert g == nv
        slot_groups.append(groups)

    with tile.TileContext(nc) as tc:
        with (
            tc.tile_pool(name="weights", bufs=1) as wpool,
            tc.tile_pool(name="exps", bufs=3) as epool,
            tc.tile_pool(name="ot", bufs=2) as otpool,
            tc.tile_pool(name="recip", bufs=2) as rpool,
            tc.tile_pool(name="outs", bufs=2) as opool,
            tc.tile_pool(name="ps_s", bufs=2, space="PSUM") as ps_s,
            tc.tile_pool(name="ps_o", bufs=1, space="PSUM") as ps_o,
            tc.tile_pool(name="ps_t", bufs=1, space="PSUM") as ps_t,
        ):
            ident = wpool.tile([128, 128], F32, tag="ident")
            make_identity(nc, ident)

            # PE warm-up: dummy matmuls on the identity while the first input
            # DMA is in flight, so the first real mm1s run at full clock
            # (HAM ramps only after sustained PE activity).
            warm = ps_t.tile([128, 128], F32, tag="pt", name="warm")
            for _ in range(6):
                nc.tensor.matmul(warm, lhsT=ident, rhs=ident, start=True, stop=True)

            # Input loads (valid prefix only), in consumption order; the two
            # DMAs the first matmul group needs go out on BOTH HWDGE rings
            # (SP + ACT) in parallel to shorten the critical head path.
            kta_s = [
                wpool.tile(
                    [D + 1, nv_slots[s] * KT], BF16, tag=f"kta{s}", name=f"kta{s}"
                )
                for s in range(NSLOT)
            ]
            qta_s = [
                wpool.tile([D + 1, QT], BF16, tag=f"qta{s}", name=f"qta{s}")
                for s in range(NSLOT)
            ]
            vaug_s = [
                wpool.tile(
                    [128, nv_slots[s] * (D + 1)], BF16, tag=f"vaug{s}", name=f"vaug{s}"
                )
                for s in range(NSLOT)
            ]
            # Process slots big/small interleaved: small slots are dominated
            # by their finish chains (po/pt are single-buffered), which then
            # hide under the neighboring big slots' long ACT windows instead
            # of piling up serially at the kernel tail.  End on the smallest
            # slot so the exposed final compute+finish chain is minimal.
            proc_order = [0, 5, 1, 6, 2, 4, 3, 7][:NSLOT]

            c0 = slot_groups[0][0][1] * KT
            nc.sync.dma_start(out=kta_s[0][:, :c0], in_=kta_d[0, :, :c0])
            nc.scalar.dma_start(out=qta_s[0], in_=qta_d[0])
            nc.sync.dma_start(
                out=kta_s[0][:, c0:], in_=kta_d[0, :, c0 : nv_slots[0] * KT]
            )
            nc.sync.dma_start(out=vaug_s[0], in_=vau_d[:, : nv_slots[0] * (D + 1)])
            for s in proc_order[1:]:
                nc.sync.dma_start(out=qta_s[s], in_=qta_d[s])
                nc.sync.dma_start(out=kta_s[s], in_=kta_d[s, :, : nv_slots[s] * KT])
                nc.sync.dma_start(
                    out=vaug_s[s],
                    in_=vau_d[:, s * NKT * (D + 1) : (s * NKT + nv_slots[s]) * (D + 1)],
                )

            def finish(s, po):
                """Normalize po [65, 512] and store as out[s]."""
                ot = otpool.tile([D + 1, QT], F32, tag="ot", name="ot")
                nc.vector.tensor_copy(ot, po)
                pt = ps_t.tile([128, QT // 128, D + 1], F32, tag="pt", name="pt")
                for j in range(QT // 128):
                    nc.tensor.transpose(
                        pt[:, j, :],
                        ot[:, j * 128 : (j + 1) * 128],
                        ident[: D + 1, : D + 1],
                    )
                rc = rpool.tile([128, QT // 128], F32, tag="rc", name="rc")
                nc.vector.reciprocal(rc, pt[:, :, D])
                ob = opool.tile([128, QT // 128, D], F32, tag="ob", name="ob")
                for j in range(QT // 128):
                    nc.vector.tensor_scalar_mul(
                        ob[:, j, :], pt[:, j, :D], rc[:, j : j + 1]
                    )
                out_ap = out_d[s].rearrange("(j p) d -> p j d", p=128)
                nc.sync.dma_start(out=out_ap, in_=ob)

            # finish() emission is deferred past the NEXT slot's first group so
            # its PE-transposes don't sit between a slot's last activation and
            # the next slot's first mm1 group in the PE stream.
            pending_finish = None
            for s in proc_order:
                nv = nv_slots[s]
                exps = epool.tile([128, nv * QT], BF16, tag="exps", name="exps")
                po = ps_o.tile([D + 1, QT], F32, tag="po", name="po")
                for gi, (g, w) in enumerate(slot_groups[s]):
                    st = ps_s.tile([128, GROUP * QT], F32, tag="st", name="st")
                    for j in range(w):
                        n = g + j
                        nc.tensor.matmul(
                            st[:, j * QT : (j + 1) * QT],
                            lhsT=kta_s[s][:, n * KT : (n + 1) * KT],
                            rhs=qta_s[s],
                            start=True,
                            stop=True,
                        )
                    nc.scalar.activation(
                        out=exps[:, g * QT : (g + w) * QT],
                        in_=st[:, : w * QT],
                        func=mybir.ActivationFunctionType.Exp,
                        scale=0.125,
                    )
                    for j in range(w):
                        n = g + j
                        nc.tensor.matmul(
                            po,
                            lhsT=vaug_s[s][:, n * (D + 1) : (n + 1) * (D + 1)],
                            rhs=exps[:, n * QT : (n + 1) * QT],
                            start=(n == 0),
                            stop=(n == nv - 1),
                            skip_group_check=True,
                        )
                    if gi == 0 and pending_finish is not None:
                        finish(*pending_finish)
                pending_finish = (s, po)
            finish(*pending_finish)

    nc.compile()
    return nc


def _plan(valid_lens):
    """Sort the 64 (batch, q-quarter) units by valid k-tile count and deal
    them into NSLOT slots of one unit per core.  Returns (core_units,
    nv_slots) where core_units[c][s] = (batch, quarter)."""
    VL = np.asarray(valid_lens).astype(np.int64)
    nv = np.maximum(1, np.minimum(NKT, (VL + KT - 1) // KT))
    qpb = LQ // QT  # quarters per batch
    unit_nv = np.repeat(nv, qpb)
    order = np.argsort(-unit_nv, kind="stable")
    core_units = [
        [(int(order[NSLOT * s + c]) // qpb, int(order[NSLOT * s + c]) % qpb) for s in range(NSLOT)]
        for c in range(N_CORES)
    ]
    nv_slots = tuple(int(unit_nv[order[NSLOT * s]]) for s in range(NSLOT))
    return core_units, nv_slots


def _shard_inputs(queries, keys, values, valid_lens, core_units):
    """Host-side layout per core: stacked per-unit augmented operands."""
    Q = np.asarray(queries, dtype=np.float32)
    K = np.asarray(keys, dtype=np.float32)
    V = np.asarray(values, dtype=np.float32)
    VL = np.asarray(valid_lens).astype(np.int64)

    cols = np.arange(LKV, dtype=np.int64)
    ones_row = np.ones((1, QT), np.float32)
    in_maps = []
    for c in range(N_CORES):
        qta = np.empty((NSLOT, D + 1, QT), np.float32)
        kta = np.empty((NSLOT, D + 1, LKV), np.float32)
        va = np.empty((128, NSLOT * NKT * (D + 1)), np.float32)
        for s, (b, qt) in enumerate(core_units[c]):
            qta[s] = np.concatenate(
                [Q[b, qt * QT : (qt + 1) * QT, :].T, ones_row], axis=0
            )
            mask = np.where(cols >= VL[b], MASK_RAW, 0.0).astype(np.float32)
            kta[s] = np.concatenate([K[b].T, mask[None, :]], axis=0)
            vb = np.concatenate([V[b], np.ones((LKV, 1), np.float32)], axis=-1)
            va[:, s * NKT * (D + 1) : (s + 1) * NKT * (D + 1)] = (
                vb.reshape(NKT, KT, D + 1).transpose(1, 0, 2).reshape(128, -1)
            )
        in_maps.append(
            {
                "qta": qta.astype(ml_dtypes.bfloat16),
                "kta": kta.astype(ml_dtypes.bfloat16),
                "vaug": va.astype(ml_dtypes.bfloat16),
            }
        )
    return in_maps


def kernel(queries, keys, values, valid_lens):
    core_units, nv_slots = _plan(valid_lens)
    nc = _build_module(nv_slots)
    in_maps = _shard_inputs(queries, keys, values, valid_lens, core_units)
    res = bass_utils.run_bass_kernel_spmd(nc, in_maps, core_ids=list(range(N_CORES)))
    out = np.empty((B, LQ, D), np.float32)
    for c in range(N_CORES):
        o = res.results[c]["o"].reshape(NSLOT, QT, D)
        for s, (b, qt) in enumerate(core_units[c]):
            out[b, qt * QT : (qt + 1) * QT, :] = o[s]
    return out


# revision 31
# speedup vs baseline: 1.2329x; 1.2329x over previous
"""Fused masked-softmax attention (DotProductAttention) for 8 TRN2 NeuronCores.

Problem: B=16 batches of Q[2048,64] @ K[2048,64]^T -> mask cols >= valid_len
to -1e6 -> softmax -> @ V[2048,64].

Work decomposition: each batch splits into 4 q-quarters of 512 rows (one
PSUM-bank-wide q-tile each) -> 64 independent units.  Units are sorted by
valid k-tile count nv = ceil(valid_len/128) and dealt into 8 SPMD slots of
8 units (one per core); the compiled program runs slot s with a static
nv_s = max over that slot's units.  K-tiles wholly past a unit's valid_len
contribute exactly 0 (the mask row drives exp to underflow), so the extra
tiles cores run inside a slot are harmless and skipped tiles are exact.
For uniform-random valid_lens this cuts total work to ~50-60% of dense;
worst case (all full) equals the dense kernel.

Per-unit kernel (all on-chip, scores never touch HBM):
  * Layout: S^T[k, q] so softmax's k-reduction becomes a matmul and the
    attn @ V contraction needs no transpose of the big matrix.
  * mm1:  S^T chunk [128k, 512q] = kTa[:, ktile].T @ qTa with AUGMENTED
    bf16 operands: kTa = [K^T; mask_row] (65 rows), qTa = [Q^T; ones].
    The 65th contraction row adds -8e6 to every masked column, so masking
    costs zero instructions.  bf16 streams 1 row/cycle on the PE (fp32 is
    4x, fp32r 2x).
  * exp:  ACT engine (the bottleneck, 1 elem/lane/cycle @1.2GHz),
    exp(0.125 * x) straight out of PSUM in merged N<=1536 activations
    (per-instruction overhead ~350 cycles), bf16 out.
  * mm2:  O^T_aug [65, 512q] = sum_k Vaug[ktile].T @ expS^T[ktile] with
    Vaug = [V | ones] (bf16) -> row 64 accumulates the softmax denominator
    in fp32 PSUM.  Interleaved group-by-group with mm1/exp.
  * finish: copy PSUM->SBUF (f32), PE-transpose 128-col chunks back to
    [q, d] layout, reciprocal of denominator column, per-partition scale,
    one merged DMA per unit.
"""

import functools

import numpy as np
import ml_dtypes

import concourse.bacc as bacc
import concourse.tile as tile
from concourse import mybir
from concourse import bass_utils
from concourse.masks import make_identity

B, LQ, LKV, D = 16, 2048, 2048, 64
N_CORES = 8
KT = 128            # k-tile (partition dim of S^T)
QT = 512            # q-rows per unit (= PSUM bank free dim)
NKT = LKV // KT     # 16
NSLOT = (B * LQ) // (N_CORES * QT)  # 8 units per core
GROUP = 3           # max k-tiles per PSUM tile / merged activation
MASK_RAW = -8.0e6   # * 0.125 scale == -1e6 (reference MASK_VALUE)
F32 = mybir.dt.float32
BF16 = mybir.dt.bfloat16


def _widths(nv):
    """Split nv k-tiles into activation groups of width <=3, avoiding 1-wide
    groups (measured regression) where possible.  2-wide groups go FIRST so
    each slot's first activation has the shortest possible mm1 prefix
    (shrinks the ACT stall at slot boundaries)."""
    threes, rem = divmod(nv, 3)
    if rem == 0:
        return [3] * threes
    if rem == 2:
        return [2] + [3] * threes
    if threes >= 1:
        return [2, 2] + [3] * (threes - 1)
    return [1]


@functools.lru_cache(maxsize=4)
def _build_module(nv_slots):
    nc = bacc.Bacc(None)
    qta_d = nc.dram_tensor("qta", [NSLOT, D + 1, QT], BF16, kind="ExternalInput")
    kta_d = nc.dram_tensor("kta", [NSLOT, D + 1, LKV], BF16, kind="ExternalInput")
    vau_d = nc.dram_tensor("vaug", [128, NSLOT * NKT * (D + 1)], BF16, kind="ExternalInput")
    out_d = nc.dram_tensor("o", [NSLOT, QT, D], F32, kind="ExternalOutput")

    slot_groups = []
    for nv in nv_slots:
        groups, g = [], 0
        for w in _widths(nv):
            groups.append((g, w))
            g += w
        assert g == nv
        slot_groups.append(groups)

    with tile.TileContext(nc) as tc:
        with (
            tc.tile_pool(name="weights", bufs=1) as wpool,
            tc.tile_pool(name="exps", bufs=3) as epool,
            tc.tile_pool(name="ot", bufs=2) as otpool,
            tc.tile_pool(name="recip", bufs=2) as rpool,
            tc.tile_pool(name="outs", bufs=2) as opool,
            tc.tile_pool(name="ps_s", bufs=2, space="PSUM") as ps_s,
            tc.tile_pool(name="ps_o", bufs=1, space="PSUM") as ps_o,
            tc.tile_pool(name="ps_t", bufs=1, space="PSUM") as ps_t,
        ):
            ident = wpool.tile([128, 128], F32, tag="ident")
            make_identity(nc, ident)

            # PE warm-up: dummy matmuls on the identity while the first input
            # DMA is in flight, so the first real mm1s run at full clock
            # (HAM ramps only after sustained PE activity).
            warm = ps_t.tile([128, 128], F32, tag="pt", name="warm")
            for _ in range(6):
                nc.tensor.matmul(warm, lhsT=ident, rhs=ident, start=True, stop=True)

            # Input loads (valid prefix only), in consumption order; the two
            # DMAs the first matmul group needs go out on BOTH HWDGE rings
            # (SP + ACT) in parallel to shorten the critical head path.
            kta_s = [
                wpool.tile(
                    [D + 1, nv_slots[s] * KT], BF16, tag=f"kta{s}", name=f"kta{s}"
                )
                for s in range(NSLOT)
            ]
            qta_s = [
                wpool.tile([D + 1, QT], BF16, tag=f"qta{s}", name=f"qta{s}")
                for s in range(NSLOT)
            ]
            vaug_s = [
                wpool.tile(
                    [128, nv_slots[s] * (D + 1)], BF16, tag=f"vaug{s}", name=f"vaug{s}"
                )
                for s in range(NSLOT)
            ]
            # Process slots big/small interleaved: small slots are dominated
            # by their finish chains (po/pt are single-buffered), which then
            # hide under the neighboring big slots' long ACT windows instead
            # of piling up serially at the kernel tail.  End on the smallest
            # slot so the exposed final compute+finish chain is minimal.
            proc_order = [0, 5, 1, 6, 2, 4, 3, 7][:NSLOT]

            c0 = slot_groups[0][0][1] * KT
            nc.sync.dma_start(out=kta_s[0][:, :c0], in_=kta_d[0, :, :c0])
            nc.scalar.dma_start(out=qta_s[0], in_=qta_d[0])
            nc.sync.dma_start(
                out=kta_s[0][:, c0:], in_=kta_d[0, :, c0 : nv_slots[0] * KT]
            )
            nc.sync.dma_start(out=vaug_s[0], in_=vau_d[:, : nv_slots[0] * (D + 1)])
            for s in proc_order[1:]:
                nc.sync.dma_start(out=qta_s[s], in_=qta_d[s])
                nc.sync.dma_start(out=kta_s[s], in_=kta_d[s, :, : nv_slots[s] * KT])
                nc.sync.dma_start(
                    out=vaug_s[s],
                    in_=vau_d[:, s * NKT * (D + 1) : (s * NKT + nv_slots[s]) * (D + 1)],
                )

            def finish(s, po):
                """Normalize po [65, 512] and store as out[s]."""
                ot = otpool.tile([D + 1, QT], F32, tag="ot", name="ot")
                nc.vector.tensor_copy(ot, po)
                pt = ps_t.tile([128, QT // 128, D + 1], F32, tag="pt", name="pt")
                for j in range(QT // 128):
                    nc.tensor.transpose(
                        pt[:, j, :],
                        ot[:, j * 128 : (j + 1) * 128],
                        ident[: D + 1, : D + 1],
                    )
                rc = rpool.tile([128, QT // 128], F32, tag="rc", name="rc")
                nc.vector.reciprocal(rc, pt[:, :, D])
                ob = opool.tile([128, QT // 128, D], F32, tag="ob", name="ob")
                for j in range(QT // 128):
                    nc.vector.tensor_scalar_mul(
                        ob[:, j, :], pt[:, j, :D], rc[:, j : j + 1]
                    )
                out_ap = out_d[s].rearrange("(j p) d -> p j d", p=128)
                nc.sync.dma_start(out=out_ap, in_=ob)

            for s in proc_order:
                nv = nv_slots[s]
                exps = epool.tile([128, nv * QT], BF16, tag="exps", name="exps")
                po = ps_o.tile([D + 1, QT], F32, tag="po", name="po")
                for g, w in slot_groups[s]:
                    st = ps_s.tile([128, GROUP * QT], F32, tag="st", name="st")
                    for j in range(w):
                        n = g + j
                        nc.tensor.matmul(
                            st[:, j * QT : (j + 1) * QT],
                            lhsT=kta_s[s][:, n * KT : (n + 1) * KT],
                            rhs=qta_s[s],
                            start=True,
                            stop=True,
                        )
                    nc.scalar.activation(
                        out=exps[:, g * QT : (g + w) * QT],
                        in_=st[:, : w * QT],
                        func=mybir.ActivationFunctionType.Exp,
                        scale=0.125,
                    )
                    for j in range(w):
                        n = g + j
                        nc.tensor.matmul(
                            po,
                            lhsT=vaug_s[s][:, n * (D + 1) : (n + 1) * (D + 1)],
                            rhs=exps[:, n * QT : (n + 1) * QT],
                            start=(n == 0),
                            stop=(n == nv - 1),
                            skip_group_check=True,
                        )
                finish(s, po)

    nc.compile()
    return nc


def _plan(valid_lens):
    """Sort the 64 (batch, q-quarter) units by valid k-tile count and deal
    them into NSLOT slots of one unit per core.  Returns (core_units,
    nv_slots) where core_units[c][s] = (batch, quarter)."""
    VL = np.asarray(valid_lens).astype(np.int64)
    nv = np.maximum(1, np.minimum(NKT, (VL + KT - 1) // KT))
    qpb = LQ // QT  # quarters per batch
    unit_nv = np.repeat(nv, qpb)
    order = np.argsort(-unit_nv, kind="stable")
    core_units = [
        [(int(order[NSLOT * s + c]) // qpb, int(order[NSLOT * s + c]) % qpb) for s in range(NSLOT)]
        for c in range(N_CORES)
    ]
    nv_slots = tuple(int(unit_nv[order[NSLOT * s]]) for s in range(NSLOT))
    return core_units, nv_slots


def _shard_inputs(queries, keys, values, valid_lens, core_units):
    """Host-side layout per core: stacked per-unit augmented operands."""
    Q = np.asarray(queries, dtype=np.float32)
    K = np.asarray(keys, dtype=np.float32)
    V = np.asarray(values, dtype=np.float32)
    VL = np.asarray(valid_lens).astype(np.int64)

    cols = np.arange(LKV, dtype=np.int64)
    ones_row = np.ones((1, QT), np.float32)
    in_maps = []
    for c in range(N_CORES):
        qta = np.empty((NSLOT, D + 1, QT), np.float32)
        kta = np.empty((NSLOT, D + 1, LKV), np.float32)
        va = np.empty((128, NSLOT * NKT * (D + 1)), np.float32)
        for s, (b, qt) in enumerate(core_units[c]):
            qta[s] = np.concatenate(
                [Q[b, qt * QT : (qt + 1) * QT, :].T, ones_row], axis=0
            )
            mask = np.where(cols >= VL[b], MASK_RAW, 0.0).astype(np.float32)
            kta[s] = np.concatenate([K[b].T, mask[None, :]], axis=0)
            vb = np.concatenate([V[b], np.ones((LKV, 1), np.float32)], axis=-1)
            va[:, s * NKT * (D + 1) : (s + 1) * NKT * (D + 1)] = (
                vb.reshape(NKT, KT, D + 1).transpose(1, 0, 2).reshape(128, -1)
            )
        in_maps.append(
            {
                "qta": qta.astype(ml_dtypes.bfloat16),
                "kta": kta.astype(ml_dtypes.bfloat16),
                "vaug": va.astype(ml_dtypes.bfloat16),
            }
        )
    return in_maps


def kernel(queries, keys, values, valid_lens):
    core_units, nv_slots = _plan(valid_lens)
    nc = _build_module(nv_slots)
    in_maps = _shard_inputs(queries, keys, values, valid_lens, core_units)
    res = bass_utils.run_bass_kernel_spmd(nc, in_maps, core_ids=list(range(N_CORES)))
    out = np.empty((B, LQ, D), np.float32)
    for c in range(N_CORES):
        o = res.results[c]["o"].reshape(NSLOT, QT, D)
        for s, (b, qt) in enumerate(core_units[c]):
            out[b, qt * QT : (qt + 1) * QT, :] = o[s]
    return out
